# revision 48
# baseline (speedup 1.0000x reference)
"""Multi-head attention block (QKV projections + softmax attention + output
projection) for TRN2, distributed over 8 NeuronCores.

Sharding: core c handles batch b = c // 2 and head group g = c % 2 (8 of
the 16 heads).  Data parallel over batch, tensor parallel over heads.
Each core computes a partial output (its head group's contribution to the
final projection); the host upcasts and sums the two bf16 partials per
batch and adds the output bias.

Layout strategy (all matmuls contract over the SBUF partition dim):
  - All matmul operands are bf16 (host casts); PSUM accumulates fp32.
    bf16 halves DMA bytes and SBUF footprint and enables FWL weight
    loads vs the fp32 path.  Measured end-to-end relative error ~5e-3.
  - Host sends q/k/v pre-transposed (qT = q[b].T, [D, n]) so projections
    need no on-device transposes.
  - Q'T, K'T are produced head-major-transposed ([j, n], j = head*64+d)
    directly by the projection matmuls.
  - Scores are computed transposed (S^T[k, q]) with head pairs row-tiled
    in the PE array (even head on partitions 0-63, odd on 64-127, running
    concurrently); exp runs on the scalar engine PSUM->SBUF writing bf16
    P~ tiles (the scalar engine's ~72us of exp is the second-longest
    engine load after the PE's ~96us of matmul slots).
  - P@V is column-tiled per head pair: even head -> po1[0:64], odd head
    -> po1[64:128] run concurrently in the PE array, and a second
    col-tiled pair with an all-ones [128,64] stationary accumulates both
    softmax denominators stacked in po2 for the same cost in PE slots.
    One [128,512] DVE reciprocal then covers the whole pair and the
    normalization multiplies are partition-aligned; they run on the
    otherwise-idle gpsimd engine off SBUF copies (the copies release the
    PSUM accumulators in ~1us so the next P@V never waits on the 3.2us
    iterative reciprocal).

Scheduling: score blocks are pipelined into the projection phase (K
projection j-chunks interleave the previous head pair's score/exp steps;
the V projection interleaves the last qc=0 score block), the drain fuses
each qc=1 score block with qc=0 P@Vs -- carrying TWO P@Vs per cycle once
qc=1 exps are resident so no P@V work piles up after the last exp -- and
the output projection fills the exp-gated tail.  The last P@V pair skips
the decoupling copies (tail latency beats PSUM release there).

The attention scale (all_head_dim**-0.5) and its effect on bq are folded
into the Q weights on the host.  The attention mask is applied as an
additive score bias only when it is not all-ones; biases are applied (as
rank-1 matmul accumulands) only when nonzero.
"""

from contextlib import ExitStack

import numpy as np
import ml_dtypes

import concourse.bass as bass
import concourse.tile as tile
from concourse import bacc, mybir
from concourse.bass_utils import run_bass_kernel_spmd

# Problem shape (hardcoded per the harness contract).
NUM_HEADS = 16
B = 4
N = 1024          # sequence length (n_q == n_k)
D = 1024          # model dim
OUT = 1024        # output dim
HD = 64           # head dim
NH = 8            # heads per core (16 heads / 2 groups)
J = NH * HD       # per-core all-head dim = 512
P = 128           # SBUF partitions
ND = D // P       # 8 contraction chunks over D
NKT = N // P      # 8 key-token chunks
NJ = J // P       # 4 chunks over j
QW = 512          # matmul moving width / PSUM bank width (fp32)
NQC = N // QW     # 2 query-column halves

F32 = mybir.dt.float32
BF16 = mybir.dt.bfloat16
NPBF16 = ml_dtypes.bfloat16


def _build(use_mask: bool, use_bias: bool, reps: int = 1):
    nc = bacc.Bacc(None, target_bir_lowering=False)

    def din(nm, shape, dt=BF16):
        return nc.declare_dram_parameter(nm, shape, dt, isOutput=False)

    qt_d, kt_d, vt_d = din("qt", [D, N]), din("kt", [D, N]), din("vt", [D, N])
    wq_d, wk_d, wv_d = din("wq", [D, J]), din("wk", [D, J]), din("wv", [D, J])
    wo_d = din("wo", [J, OUT])
    if use_bias:
        bqs_d, bks_d, bvs_d = (
            din("bqs", [1, J]), din("bks", [1, J]), din("bvs", [1, J])
        )
    mb_d = din("mb", [N, N], F32) if use_mask else None
    out_d = nc.declare_dram_parameter("out", [N, OUT], BF16, isOutput=True)

    AF = mybir.ActivationFunctionType
    big_bufs = 20 if use_mask else 26
    pt_bufs = 28 if use_mask else 32

    with tile.TileContext(nc) as tc:
        with ExitStack() as ctx:
            # 256 KB streaming slots for input chunks; P^T (exp) tiles get
            # their own pool so they never contend with stream tiles (that
            # contention deadlocks: exp -> stream-slot -> proj-MM -> psum
            # rotation -> ACT FIFO -> exp).
            big = ctx.enter_context(tc.tile_pool(name="big", bufs=big_bufs))
            ptp = ctx.enter_context(tc.tile_pool(name="ptp", bufs=pt_bufs))
            pacts = ctx.enter_context(tc.tile_pool(name="acts", bufs=1))
            pwo = ctx.enter_context(tc.tile_pool(name="pwo", bufs=1))
            pout = ctx.enter_context(tc.tile_pool(name="outc", bufs=2))
            psml = ctx.enter_context(tc.tile_pool(name="small", bufs=1))
            # PSUM: tag "ps" = 3 x [P,2,QW] (6 banks; projections, scores,
            # finals share the rotation), tag "po" = 2 x [P,QW] (2 banks).
            psum = ctx.enter_context(tc.tile_pool(name="ps", bufs=3, space="PSUM"))
            if use_mask:
                pmask = ctx.enter_context(tc.tile_pool(name="pmask", bufs=1))

            # Small persistent tiles.
            onesf = psml.tile([1, QW], F32, name="onesf", tag="onesf")
            nc.vector.memset(onesf[:], 1.0)
            if use_bias:
                ones_row = psml.tile([1, QW], BF16, name="ones_row", tag="ones_row")
                nc.vector.tensor_copy(ones_row[:], onesf[:])
                bqs_t = psml.tile([1, J], BF16, name="bqs_t", tag="bqs")
                bks_t = psml.tile([1, J], BF16, name="bks_t", tag="bks")
                bvs_t = psml.tile([1, J], BF16, name="bvs_t", tag="bvs")
                nc.sync.dma_start(bqs_t[:], bqs_d[:])
                nc.sync.dma_start(bks_t[:], bks_d[:])
                nc.sync.dma_start(bvs_t[:], bvs_d[:])

            # Warm the ACT exp table while DMAs run.
            warm = psml.tile([1, QW], F32, name="warm", tag="rc", bufs=2)
            nc.scalar.activation(warm[:], onesf[:], AF.Exp)

            if use_mask:
                mb_t = pmask.tile([P, NKT, N], F32, name="mb_t", tag="mask")
                nc.sync.dma_start(
                    mb_t[:], mb_d[:].rearrange("(a p) n -> p a n", p=P)
                )

            def _emit_rep():
                # --- streamed input chunks (256 KB each), first-use order.
                # w chunks: [P, 2, J] covering 2 D-chunks; x chunks: [P, N]
                # covering 1 D-chunk.
                def load_wx(wd, xd, nm):
                    w, x = [], []
                    for i in range(4):
                        t = big.tile([P, 2, J], BF16, name=f"w{nm}{i}", tag="big")
                        w.append(t)
                    for i in range(ND):
                        t = big.tile([P, N], BF16, name=f"x{nm}{i}", tag="big")
                        x.append(t)
                    order = [(w, 0), (x, 0), (x, 1), (w, 1), (x, 2), (x, 3),
                             (w, 2), (x, 4), (x, 5), (w, 3), (x, 6), (x, 7)]
                    for lst, i in order:
                        if lst is w:
                            nc.sync.dma_start(
                                w[i][:],
                                wd[i * (2 * P):(i + 1) * (2 * P), :].rearrange(
                                    "(a p) j -> p a j", p=P
                                ),
                            )
                        else:
                            nc.sync.dma_start(x[i][:], xd[i * P:(i + 1) * P, :])
                    return w, x

                def wslice(w, dc, cols):
                    return w[dc // 2][:, dc % 2, cols]

                qpt = pacts.tile([P, NJ, N], BF16, name="qpt", tag="qpt")
                kpt = pacts.tile([P, NJ, N], BF16, name="kpt", tag="kpt")
                vext = pacts.tile([P, NKT, NH, HD], BF16, name="vext",
                                  tag="vext")
                ot = pacts.tile([P, NJ, N], BF16, name="ot", tag="ot")

                # all-ones stationary for the denominator matmuls
                ones64 = psml.tile([P, HD], BF16, name="ones64", tag="ones64")
                nc.vector.memset(ones64[:], 1.0)

                # --- Q/K projection: contraction-chunk outer over
                # concurrently accumulating PSUM banks.
                def proj_group(cs, w, x, bias_t, dst, steps=None):
                    groups = {
                        c: psum.tile([P, NQC, QW], F32, name="psp", tag="ps")
                        for c in cs
                    }
                    if use_bias:
                        for c in cs:
                            for qc in range(NQC):
                                nc.tensor.matmul(
                                    groups[c][:, qc, :],
                                    bias_t[0:1, c * P:(c + 1) * P],
                                    ones_row[:], start=True, stop=False,
                                    skip_group_check=True,
                                )
                    for dc in range(ND):
                        for c in cs:
                            for qc in range(NQC):
                                nc.tensor.matmul(
                                    groups[c][:, qc, :],
                                    wslice(w, dc, slice(c * P, (c + 1) * P)),
                                    x[dc][:, qc * QW:(qc + 1) * QW],
                                    start=(dc == 0 and not use_bias),
                                    stop=(dc == ND - 1),
                                    skip_group_check=True,
                                )
                        if steps is not None and dc % 2 == 1:
                            steps[dc // 2]()
                    for c in cs:
                        nc.vector.tensor_copy(
                            dst[:, c, :],
                            groups[c][:].rearrange("p a q -> p (a q)"),
                        )

                # --- score block steps: one closure per kc-pair; each does
                # the 4 score matmuls (2 kc x 2 row-tiled heads) + 2 exps.
                pts = {}

                def s_block_steps(pr, qc):
                    for h in (2 * pr, 2 * pr + 1):
                        pts[(h, qc)] = [None] * (NKT // 2)

                    def step(kcp):
                        pss = {}
                        for h in (2 * pr, 2 * pr + 1):
                            pts[(h, qc)][kcp] = ptp.tile(
                                [P, 2, QW], BF16, name="pt", tag="pt"
                            )
                            pss[h] = psum.tile([P, 2, QW], F32, name="pss",
                                               tag="ps")
                        for i in range(2):
                            kc = 2 * kcp + i
                            for h in (2 * pr, 2 * pr + 1):
                                off = HD * (h & 1)
                                nc.tensor.matmul(
                                    pss[h][:, i, :],
                                    kpt[off:off + HD, pr, kc * P:(kc + 1) * P],
                                    qpt[off:off + HD, pr, qc * QW:(qc + 1) * QW],
                                    start=True, stop=True,
                                    skip_group_check=True,
                                )
                        for h in (2 * pr, 2 * pr + 1):
                            if use_mask:
                                nc.vector.tensor_add(
                                    pss[h][:],
                                    pss[h][:],
                                    mb_t[:, 2 * kcp:2 * kcp + 2,
                                         qc * QW:(qc + 1) * QW],
                                )
                            nc.scalar.activation(
                                pts[(h, qc)][kcp][:], pss[h][:], AF.Exp,
                            )

                    return [lambda kcp=kcp: step(kcp)
                            for kcp in range(NKT // 2)]

                # --- P@V steps for one head pair: accumulate
                # po[0:64]  = O~^T (unnormalized), po[64:128] = denominator
                # replicated; finish() normalizes into ot via a wide
                # reciprocal + multiply.
                def pv_steps(task, last=False):
                    # Col-tiled P@V: even head -> po1[0:64], odd head ->
                    # po1[64:128] (concurrent col-tiles), plus an all-ones
                    # stationary pair producing both softmax denominators
                    # stacked in po2 -- one [128,512] reciprocal then
                    # normalizes the whole head pair.
                    prv, qcv = task
                    pta = pts.pop((2 * prv, qcv))
                    ptb = pts.pop((2 * prv + 1, qcv))
                    po1 = psum.tile([P, QW], F32, name="po", tag="po", bufs=2)
                    po2 = psum.tile([P, QW], F32, name="po", tag="po", bufs=2)

                    def step(kcp):
                        for i in range(2):
                            kc = 2 * kcp + i
                            se = (kc == 0)
                            sp = (kc == NKT - 1)
                            for half, pt_, hv in ((0, pta, 2 * prv),
                                                  (1, ptb, 2 * prv + 1)):
                                mv = pt_[kc // 2][:, kc % 2, :]
                                nc.tensor.matmul(
                                    po1[half * HD:(half + 1) * HD, :],
                                    vext[:, kc, hv, :], mv,
                                    start=se, stop=sp,
                                    tile_position=(0, half * HD),
                                    skip_group_check=True,
                                )
                                nc.tensor.matmul(
                                    po2[half * HD:(half + 1) * HD, :],
                                    ones64[:], mv,
                                    start=se, stop=sp,
                                    tile_position=(0, half * HD),
                                    skip_group_check=True,
                                )

                    def finish():
                        rb = psml.tile([P, QW], F32, name="rb", tag="rb",
                                       bufs=3)
                        if last:
                            # tail latency matters more than PSUM release:
                            # run the whole chain directly off PSUM
                            nc.vector.reciprocal(rb[:], po2[:])
                            for half, hv in ((0, 2 * prv), (1, 2 * prv + 1)):
                                off = HD * (hv & 1)
                                nc.vector.tensor_mul(
                                    ot[off:off + HD, hv // 2,
                                       qcv * QW:(qcv + 1) * QW],
                                    po1[half * HD:(half + 1) * HD, :],
                                    rb[half * HD:(half + 1) * HD, :],
                                )
                            return
                        # Copy both accumulators to SBUF first so the PSUM
                        # banks release in ~1us (the 3.2us iterative
                        # reciprocal then runs off the PV critical path).
                        poc = psml.tile([P, QW], F32, name="poc", tag="poc",
                                        bufs=3)
                        dnm = psml.tile([P, QW], F32, name="dnm", tag="dnm",
                                        bufs=2)
                        nc.vector.tensor_copy(poc[:], po1[:])
                        nc.vector.tensor_copy(dnm[:], po2[:])
                        nc.vector.reciprocal(rb[:], dnm[:])
                        # normalization multiplies on the otherwise-idle
                        # gpsimd engine (SBUF-only operands) to keep DVE
                        # under the exp period in double-PV cycles
                        for half, hv in ((0, 2 * prv), (1, 2 * prv + 1)):
                            off = HD * (hv & 1)
                            nc.gpsimd.tensor_mul(
                                ot[off:off + HD, hv // 2,
                                   qcv * QW:(qcv + 1) * QW],
                                poc[half * HD:(half + 1) * HD, :],
                                rb[half * HD:(half + 1) * HD, :],
                            )

                    return [lambda kcp=kcp: step(kcp)
                            for kcp in range(NKT // 2)], finish

                # --- Q projection (no interleave: DMA-paced).
                qw_, qx = load_wx(wq_d, qt_d, "q")
                proj_group(range(3), qw_, qx,
                           bqs_t if use_bias else None, qpt)
                proj_group(range(3, NJ), qw_, qx,
                           bqs_t if use_bias else None, qpt)

                # --- K projection, one j-chunk at a time, each fused with
                # the previous head pair's score block so the scalar engine
                # starts exp work as early as possible.
                kw, kx = load_wx(wk_d, kt_d, "k")
                for c in range(NJ):
                    steps = s_block_steps(c - 1, 0) if c > 0 else None
                    proj_group([c], kw, kx,
                               bks_t if use_bias else None, kpt, steps=steps)

                # --- V projection (into [k, j] + ones block) in two passes
                # over 3 two-bank PSUM tiles, with the last qc=0 score
                # block's steps interleaved so ACT stays fed.
                vw, vx = load_wx(wv_d, vt_d, "v")
                s3 = s_block_steps(NJ - 1, 0)
                for kcs, steps in ((range(0, 6), s3), (range(6, NKT), None)):
                    vgroups = {
                        kc: psum.tile([P, 2, QW], F32, name="psv", tag="ps")
                        for kc in kcs[::2]
                    }
                    if use_bias:
                        for kc in kcs:
                            nc.tensor.matmul(
                                vgroups[kc - kc % 2][:, kc % 2, :],
                                ones_row[0:1, 0:P], bvs_t[:],
                                start=True, stop=False,
                                skip_group_check=True,
                            )
                    for dc in range(ND):
                        for kc in kcs:
                            nc.tensor.matmul(
                                vgroups[kc - kc % 2][:, kc % 2, :],
                                vx[dc][:, kc * P:(kc + 1) * P],
                                wslice(vw, dc, slice(0, J)),
                                start=(dc == 0 and not use_bias),
                                stop=(dc == ND - 1),
                                skip_group_check=True,
                            )
                        if steps is not None and dc % 2 == 1:
                            steps[dc // 2]()
                    for kc in kcs[::2]:
                        nc.vector.tensor_copy(
                            vext[:, kc:kc + 2, :, 0:HD],
                            vgroups[kc][:].rearrange(
                                "p a (h d) -> p a h d", h=NH
                            ),
                        )

                wo_t = pwo.tile([P, NJ, OUT], BF16, name="wo_t", tag="wo")
                nc.sync.dma_start(
                    wo_t[:], wo_d[:].rearrange("(a p) n -> p a n", p=P)
                )

                # --- Output projection (partial over this core's heads),
                # per 128-query-row chunk.
                def emit_final_partial(qm, jcs):
                    ps = psum.tile([P, NQC, QW], F32, name="psf", tag="ps")
                    for oc in range(NQC):
                        for jc in jcs:
                            nc.tensor.matmul(
                                ps[:, oc, :],
                                ot[:, jc, qm * P:(qm + 1) * P],
                                wo_t[:, jc, oc * QW:(oc + 1) * QW],
                                start=(jc == 0), stop=(jc == NJ - 1),
                                skip_group_check=True,
                            )
                    return ps

                def emit_final_finish(qm, ps, jcs, cast_eng=None):
                    for oc in range(NQC):
                        for jc in jcs:
                            nc.tensor.matmul(
                                ps[:, oc, :],
                                ot[:, jc, qm * P:(qm + 1) * P],
                                wo_t[:, jc, oc * QW:(oc + 1) * QW],
                                start=(jc == 0), stop=(jc == NJ - 1),
                                skip_group_check=True,
                            )
                    oc_t = pout.tile([P, OUT], BF16, name="oct", tag="outc")
                    if cast_eng == "scalar":
                        # scalar engine is idle once the exps are done --
                        # the tail casts overlap DVE's last normalization
                        nc.scalar.copy(
                            oc_t[:], ps[:].rearrange("p a q -> p (a q)")
                        )
                    else:
                        nc.vector.tensor_copy(
                            oc_t[:], ps[:].rearrange("p a q -> p (a q)")
                        )
                    nc.sync.dma_start(out_d[qm * P:(qm + 1) * P, :], oc_t[:])

                def emit_final_qm(qm, cast_eng=None):
                    ps = emit_final_partial(qm, range(NJ - 1))
                    emit_final_finish(qm, ps, range(NJ - 1, NJ), cast_eng)

                # --- drain: each qc=1 score block fused with a qc=0 P@V
                # at kc-pair granularity so the PE has ready P@V work while
                # score matmuls wait on exp slot releases.
                def fused(cur, pv_tasks, last=False):
                    ssteps = s_block_steps(*cur) if cur else [None] * 4
                    pv = [pv_steps(t, last=last) for t in pv_tasks]
                    for kcp in range(NKT // 2):
                        if ssteps[kcp] is not None:
                            ssteps[kcp]()
                        for psteps, _ in pv:
                            psteps[kcp]()
                    for _, pfin in pv:
                        pfin()

                # Late drain cycles carry TWO P@Vs (a qc=0 one plus a qc=1
                # one whose exps finished two cycles earlier) so the P@V
                # work doesn't pile up after the last exp.
                fused((0, 1), [(0, 0)])
                fused((1, 1), [(1, 0)])
                fused((2, 1), [(2, 0), (0, 1)])
                fused((3, 1), [(3, 0), (1, 1)])
                fused(None, [(2, 1)])
                emit_final_qm(0)
                emit_final_qm(1)
                fused(None, [(3, 1)], last=True)
                emit_final_qm(2)
                emit_final_qm(3)
                for qm in range(4, 8):
                    emit_final_qm(qm, "scalar" if qm >= 6 else None)

            if reps == 1:
                _emit_rep()
            else:
                with tc.For_i(0, reps, 1):
                    _emit_rep()

    nc.compile()
    return nc


_NC_CACHE = {}


def _get_nc(use_mask: bool, use_bias: bool = False, reps: int = 1):
    key = (use_mask, use_bias, reps)
    if key not in _NC_CACHE:
        _NC_CACHE[key] = _build(use_mask, use_bias, reps)
    return _NC_CACHE[key]


def _group_weights(Wq, bq, Wk, bk, Wv, bv, Wo, g):
    """Per-head-group weight slices in per-core layout j = head*64 + d.

    The module splits heads as reshape(b, n, head_dim, NUM_HEADS), so
    global column d*NUM_HEADS + h belongs to (head h, dim d).
    """
    scale = float(NUM_HEADS * HD) ** -0.5
    cols = np.array(
        [d * NUM_HEADS + (NH * g + hl) for hl in range(NH) for d in range(HD)]
    )
    f = NPBF16

    return {
        "wq": np.ascontiguousarray(Wq[:, cols] * scale).astype(f),
        "bqs": np.ascontiguousarray((bq[cols] * scale)[None, :]).astype(f),
        "wk": np.ascontiguousarray(Wk[:, cols]).astype(f),
        "bks": np.ascontiguousarray(bk[cols][None, :]).astype(f),
        "wv": np.ascontiguousarray(Wv[:, cols]).astype(f),
        "bvs": np.ascontiguousarray(bv[cols][None, :]).astype(f),
        "wo": np.ascontiguousarray(Wo[cols, :]).astype(f),
    }


def make_in_maps(q, k, v, attn_mask, Wq, bq, Wk, bk, Wv, bv, Wo, bo):
    """Shard the full inputs into 8 per-core input maps."""
    use_mask = not bool(np.all(np.asarray(attn_mask) == 1.0))
    use_bias = bool(
        np.any(np.asarray(bq)) or np.any(np.asarray(bk)) or np.any(np.asarray(bv))
    )
    gw = [_group_weights(Wq, bq, Wk, bk, Wv, bv, Wo, g) for g in range(2)]
    f = NPBF16
    xt = [
        {
            "qt": np.ascontiguousarray(np.asarray(q[b]).T).astype(f),
            "kt": np.ascontiguousarray(np.asarray(k[b]).T).astype(f),
            "vt": np.ascontiguousarray(np.asarray(v[b]).T).astype(f),
        }
        for b in range(B)
    ]
    mb = None
    if use_mask:
        mb = np.ascontiguousarray(
            (-100000000.0 * (1.0 - np.asarray(attn_mask))).T, dtype=np.float32
        )
    in_maps = []
    for c in range(8):
        b, g = divmod(c, 2)
        m = dict(xt[b])
        m.update(gw[g])
        if not use_bias:
            for nm in ("bqs", "bks", "bvs"):
                m.pop(nm, None)
        if use_mask:
            m["mb"] = mb
        in_maps.append(m)
    return in_maps, use_mask, use_bias


def kernel(q, k, v, attn_mask, Wq, bq, Wk, bk, Wv, bv, Wo, bo):
    in_maps, use_mask, use_bias = make_in_maps(
        q, k, v, attn_mask, Wq, bq, Wk, bk, Wv, bv, Wo, bo
    )
    nc = _get_nc(use_mask, use_bias)
    res = run_bass_kernel_spmd(nc, in_maps, list(range(8)))
    out = np.empty((B, N, OUT), np.float32)
    bo = np.asarray(bo, np.float32)
    for b in range(B):
        out[b] = (
            res.results[2 * b]["out"].astype(np.float32)
            + res.results[2 * b + 1]["out"].astype(np.float32)
            + bo
        )
    return out
